# revision 15
# baseline (speedup 1.0000x reference)
"""KANLinear forward on 8 Trainium2 NeuronCores (Bass/Tile, SPMD data-parallel).

Math: for x in [0,1) on the uniform grid (-1,1,5) with spline order 3, the
8 B-spline basis columns span the same 6-dim space as
    {1, x, x^2, x^3, R6, R7},  R6 = relu(2.5x-0.5)^3, R7 = relu(2.5x-1.5)^3,
and silu(x) on [0,1) is approximated in the same span (max err 1.7e-5), so
BOTH branches become one dense matmul against host-refolded weights plus a
per-output bias. Device contraction: {x, x2, x3, R6, R7} -> K = 5*512 = 2560.

The PE array streams 1 element/cell/cycle regardless of dtype, so the matmul
floor is ~69us/core; everything else must hide under it. bf16 operands halve
weight/x DMA bytes, enable FWL weight loads and 2x DVE mode. All input DMAs
are single contiguous blocks (4-5KB per-partition lines) because 1KB-line
transfers measured only ~112 GB/s (descriptor-overhead-bound). The k-order is
fb-major so the first matmuls need only x_fb0 + w_fb0 (~1.1 MiB of DMA);
group 0 of each fb is the raw x tile. The last batch tile runs ob-major so
PSUM evictions overlap the final matmuls; output DMAs alternate between the
sync and scalar HWDGE queues. Output stays f32.

Sharding: batch split across 8 cores; weights replicated; x and out are
transposed host-side so features sit on the partition axis.
"""

from math import comb

import ml_dtypes
import numpy as np

BATCH = 16384
IN_F = 512
OUT_F = 512
N_CORES = 8
BS = BATCH // N_CORES        # 2048 batch rows per core
BT = 512                     # moving-dim (batch) tile
NB = BS // BT                # 4 batch tiles per core
NFB = IN_F // 128            # 4 feature blocks
NQ = 5                       # basis groups per feature: x, x2, x3, R6, R7
KT = NFB * NQ                # 20 contraction k-tiles of 128
NO = OUT_F // 128            # 4 output blocks

_CACHE = {}


def _col_coeffs():
    # Coefficients of spline columns j=0..7 over {1, d, d2, d3, R6, R7},
    # d = s - 6.75, s = 2.5x + 5.5.
    a = [1.0, -4.0, 6.0, -4.0, 1.0]
    C = np.zeros((8, 6))
    for j in range(8):
        m = np.zeros(4)
        for k in range(5):
            p = j + k
            if p <= 5:
                e = 6.75 - p
                m += (a[k] / 6.0) * np.array([e**3, 3 * e**2, 3 * e, 1.0])
        C[j, :4] = m
        if 0 <= 6 - j <= 4:
            C[j, 4] = a[6 - j] / 6.0
        if 0 <= 7 - j <= 4:
            C[j, 5] = a[7 - j] / 6.0
    return C


def _prep_weights(base_weight, spline_weight, spline_scaler):
    C = _col_coeffs()
    # change of basis {1, d, d2, d3} -> x-monomials {1, x, x2, x3}:
    # d^m = sum_j binom(m,j) (2.5x)^j (-1.25)^(m-j)
    T = np.zeros((4, 4))
    for m in range(4):
        for j in range(m + 1):
            T[m, j] = comb(m, j) * (2.5**j) * ((-1.25) ** (m - j))
    Cx = np.zeros((8, 6))
    Cx[:, :4] = C[:, :4] @ T
    Cx[:, 4:] = C[:, 4:]
    W = spline_weight.astype(np.float64) * spline_scaler.astype(np.float64)[:, :, None]
    Wt = np.einsum("ofj,jq->ofq", W, Cx)         # (out, in, 6) over {1,x,x2,x3,R6,R7}
    # Fold the base branch in as well: silu on [0,1) fitted (max err 1.7e-5)
    # in the same 6-function span.
    xs = np.linspace(0, 1, 8193)[:-1]
    V = np.stack([np.ones_like(xs), xs, xs**2, xs**3,
                  np.maximum(2.5 * xs - 0.5, 0) ** 3,
                  np.maximum(2.5 * xs - 1.5, 0) ** 3], -1)
    coef = np.linalg.lstsq(V, xs / (1 + np.exp(-xs)), rcond=None)[0]
    Wt = Wt + base_weight.astype(np.float64)[:, :, None] * coef[None, None, :]
    bias = Wt[:, :, 0].sum(axis=1)               # (out,)
    # per-fb weight block: [128 in-features, NQ*OUT_F] with q-major columns,
    # one contiguous 640 KiB DMA per fb. Group order: x, x2, x3, R6, R7.
    wT = np.empty((NFB, 128, NQ * OUT_F), dtype=ml_dtypes.bfloat16)
    for fb in range(NFB):
        fs = slice(fb * 128, (fb + 1) * 128)
        for q in range(NQ):
            wT[fb, :, q * OUT_F:(q + 1) * OUT_F] = \
                Wt[:, fs, q + 1].T.astype(ml_dtypes.bfloat16)
    # (128, NO): column ob holds the biases for out-features ob*128..+128
    return wT, np.ascontiguousarray(bias.astype(np.float32).reshape(NO, 128).T)


def _build_program():
    if "nc" in _CACHE:
        return _CACHE["nc"]
    import concourse.bacc as bacc
    import concourse.mybir as mybir
    import concourse.tile as tile

    f32 = mybir.dt.float32
    bf16 = mybir.dt.bfloat16
    AF = mybir.ActivationFunctionType
    ALU = mybir.AluOpType

    nc = bacc.Bacc(None, target_bir_lowering=False, debug=False, num_devices=N_CORES)
    xT_d = nc.dram_tensor("xT", (NFB, 128, BS), bf16, kind="ExternalInput")
    wT_d = nc.dram_tensor("wT", (NFB, 128, NQ * OUT_F), bf16, kind="ExternalInput")
    bias_d = nc.dram_tensor("bias", (128, NO), f32, kind="ExternalInput")
    outT_d = nc.dram_tensor("outT", (OUT_F, BS), f32, kind="ExternalOutput")

    with tile.TileContext(nc) as tc:
        with (
            tc.tile_pool(name="wpool", bufs=1) as wpool,
            tc.tile_pool(name="bpool", bufs=32) as bpool,
            tc.tile_pool(name="spool", bufs=8) as spool,
            tc.tile_pool(name="opool", bufs=8) as opool,
            tc.tile_pool(name="psum", bufs=2, space="PSUM") as ppool,
        ):
            # Dummy warm-up matmuls bridge the gap between the PE's preamble
            # (~6.5us, all-engine barriers) and the first weights landing
            # (~9.7us): they keep the HAM activity window busy so the real
            # stream runs at 2.4 GHz from its first instruction.
            dummy_sb = wpool.tile([128, BT], bf16, tag="dummy")
            nc.vector.memset(dummy_sb[:], 0.0)
            dummy_ps = ppool.tile([128, BT], f32, tag="acc0", name="dummy_ps")
            for _ in range(10):
                nc.tensor.matmul(dummy_ps[:], dummy_sb[:, 0:128], dummy_sb[:],
                                 start=True, stop=True)

            # Input DMAs spread across all three provisioned queues (sync
            # HWDGE, scalar HWDGE, gpsimd SWDGE; ~165 GB/s each, HBM-capped
            # in aggregate), ordered so each block lands just before the
            # PE/DVE needs it. fb-major k-order means the critical prefix is
            # x_fb0(bt0 slice) + w_fb0 cols 0:1536.
            x_sb = [None] * NFB
            w_sb = [None] * NFB
            for fb in range(NFB):
                x_sb[fb] = wpool.tile([128, BS], bf16, tag=f"x{fb}", name=f"x{fb}")
                w_sb[fb] = wpool.tile([128, NQ * OUT_F], bf16, tag=f"w{fb}",
                                      name=f"w{fb}")
            # head uses only sync + gpsimd (a third concurrent queue measured
            # SLOWER in aggregate); scalar HWDGE only carries tail outputs.
            nc.sync.dma_start(x_sb[0][:, 0:BT], xT_d[0][:, 0:BT])
            nc.sync.dma_start(w_sb[0][:], wT_d[0])
            nc.sync.dma_start(w_sb[1][:], wT_d[1])
            nc.sync.dma_start(x_sb[0][:, BT:], xT_d[0][:, BT:])
            nc.sync.dma_start(x_sb[3][:], xT_d[3])
            nc.gpsimd.dma_start(x_sb[1][:], xT_d[1])
            nc.gpsimd.dma_start(w_sb[2][:], wT_d[2])
            nc.gpsimd.dma_start(x_sb[2][:], xT_d[2])
            nc.gpsimd.dma_start(w_sb[3][:], wT_d[3])
            bias_sb = wpool.tile([128, NO], f32, tag="bias")
            nc.gpsimd.dma_start(bias_sb[:], bias_d[:])
            cbias = {}
            for v in (-0.5, -1.5):
                ct = wpool.tile([128, 1], f32, tag=f"c{v}")
                nc.vector.memset(ct[:], v)
                cbias[v] = ct

            for bt in range(NB):
                bsl = slice(bt * BT, (bt + 1) * BT)
                basis = [None] * KT          # k = fb*NQ + q
                for fb in range(NFB):
                    xt = x_sb[fb][:, bsl]
                    x2 = bpool.tile([128, BT], bf16, tag="basis")
                    x3 = bpool.tile([128, BT], bf16, tag="basis")
                    r6 = bpool.tile([128, BT], bf16, tag="basis")
                    r7 = bpool.tile([128, BT], bf16, tag="basis")
                    u6 = spool.tile([128, BT], bf16, tag="tmp")
                    u7 = spool.tile([128, BT], bf16, tag="tmp")
                    q6 = spool.tile([128, BT], bf16, tag="tmp")
                    q7 = spool.tile([128, BT], bf16, tag="tmp")
                    # ACT: the two relu shoulders u = relu(2.5x + b)
                    nc.scalar.activation(u6[:], xt, AF.Relu, scale=2.5,
                                         bias=cbias[-0.5][:])
                    nc.scalar.activation(u7[:], xt, AF.Relu, scale=2.5,
                                         bias=cbias[-1.5][:])
                    # DVE: pure bf16 mul chains; relu(u)^3 = relu(u)^2*relu(u)
                    nc.vector.tensor_mul(x2[:], xt, xt)
                    nc.vector.tensor_mul(x3[:], x2[:], xt)
                    nc.vector.tensor_mul(q6[:], u6[:], u6[:])
                    nc.vector.tensor_mul(r6[:], q6[:], u6[:])
                    nc.vector.tensor_mul(q7[:], u7[:], u7[:])
                    nc.vector.tensor_mul(r7[:], q7[:], u7[:])
                    grp = [xt, x2[:], x3[:], r6[:], r7[:]]
                    for q in range(NQ):
                        basis[fb * NQ + q] = grp[q]
                def mm(k, ob, acc, csl=slice(0, BT)):
                    fb, q = divmod(k, NQ)
                    nc.tensor.matmul(
                        acc[:],
                        w_sb[fb][:, q * OUT_F + ob * 128:q * OUT_F + ob * 128 + 128],
                        basis[k][:, csl],
                        start=(k == 0), stop=(k == KT - 1),
                    )

                if bt < NB - 1:
                    # k-major: k=0 needs only the x_fb0 + w_fb0 DMAs
                    accs = []
                    for ob in range(NO):
                        acc = ppool.tile([128, BT], f32, tag=f"acc{ob}",
                                         name=f"acc{ob}")
                        accs.append(acc)
                    for k in range(KT):
                        for ob in range(NO):
                            mm(k, ob, accs[ob])
                    for ob in range(NO):
                        osl = slice(ob * 128, (ob + 1) * 128)
                        ot = opool.tile([128, BT], f32, tag="o")
                        nc.vector.tensor_scalar(ot[:], accs[ob][:],
                                                bias_sb[:, ob:ob + 1], None, ALU.add)
                        nc.sync.dma_start(outT_d[osl, bsl], ot[:])
                else:
                    # ob-major on the last tile: acc[ob] stops 20 MMs before
                    # acc[ob+1], so evictions overlap the remaining matmuls;
                    # the final ob runs in two batch halves to shorten the
                    # last evict+writeback chain, split across both HWDGE
                    # queues.
                    for ob in range(NO - 1):
                        acc = ppool.tile([128, BT], f32, tag=f"acc{ob}",
                                         name=f"acc{ob}")
                        for k in range(KT):
                            mm(k, ob, acc)
                        osl = slice(ob * 128, (ob + 1) * 128)
                        ot = opool.tile([128, BT], f32, tag="o")
                        nc.vector.tensor_scalar(ot[:], acc[:],
                                                bias_sb[:, ob:ob + 1], None, ALU.add)
                        eng = nc.sync if ob % 2 == 0 else nc.scalar
                        eng.dma_start(outT_d[osl, bsl], ot[:])
                    ob = NO - 1
                    for h in range(2):
                        acch = ppool.tile([128, BT // 2], f32, tag=f"acc{ob}",
                                          name=f"acc{ob}h{h}")
                        csl = slice(h * (BT // 2), (h + 1) * (BT // 2))
                        for k in range(KT):
                            mm(k, ob, acch, csl)
                        oth = opool.tile([128, BT // 2], f32, tag="o",
                                         name=f"ot{ob}h{h}")
                        nc.vector.tensor_scalar(oth[:], acch[:],
                                                bias_sb[:, ob:ob + 1], None, ALU.add)
                        eng = nc.sync if h == 0 else nc.scalar
                        eng.dma_start(
                            outT_d[ob * 128:(ob + 1) * 128,
                                   bt * BT + h * (BT // 2):
                                   bt * BT + (h + 1) * (BT // 2)],
                            oth[:])

    nc.compile()
    _CACHE["nc"] = nc
    return nc


def _make_in_maps(x, base_weight, spline_weight, spline_scaler):
    wT, bias = _prep_weights(base_weight, spline_weight, spline_scaler)
    in_maps = []
    for c in range(N_CORES):
        xs = np.ascontiguousarray(
            x[c * BS:(c + 1) * BS, :].T
        ).reshape(NFB, 128, BS).astype(ml_dtypes.bfloat16)
        in_maps.append({"xT": xs, "wT": wT, "bias": bias})
    return in_maps


def kernel(x, base_weight, spline_weight, spline_scaler):
    from concourse.bass_utils import run_bass_kernel_spmd

    nc = _build_program()
    in_maps = _make_in_maps(x, base_weight, spline_weight, spline_scaler)
    res = run_bass_kernel_spmd(nc, in_maps, list(range(N_CORES)))
    out = np.empty((BATCH, OUT_F), dtype=np.float32)
    for c in range(N_CORES):
        out[c * BS:(c + 1) * BS, :] = res.results[c]["outT"].T
    return out


# revision 19
# speedup vs baseline: 1.2064x; 1.2064x over previous
"""KANLinear forward on 8 Trainium2 NeuronCores (Bass/Tile, SPMD data-parallel).

Math: for x in [0,1) on the uniform grid (-1,1,5) with spline order 3, the
8 B-spline basis columns span the same 6-dim space as
    {1, x, x^2, x^3, R6, R7},  R6 = relu(2.5x-0.5)^3, R7 = relu(2.5x-1.5)^3,
and silu(x) on [0,1) is approximated in the same span (max err 1.7e-5), so
BOTH branches become one dense matmul against host-refolded weights plus a
per-output bias. Device contraction: {x, x2, x3, R6, R7} -> K = 5*512 = 2560.

The PE array streams 1 element/cell/cycle regardless of dtype, so the matmul
floor is ~69us/core; everything else must hide under it. bf16 operands halve
weight/x DMA bytes, enable FWL weight loads and 2x DVE mode. All input DMAs
are single contiguous blocks (4-5KB per-partition lines) because 1KB-line
transfers measured only ~112 GB/s (descriptor-overhead-bound). The k-order is
fb-major so the first matmuls need only x_fb0 + w_fb0 (~1.1 MiB of DMA);
group 0 of each fb is the raw x tile. The last batch tile runs ob-major so
PSUM evictions overlap the final matmuls; output DMAs alternate between the
sync and scalar HWDGE queues. Output stays f32.

Sharding: batch split across 8 cores; weights replicated; x and out are
transposed host-side so features sit on the partition axis.
"""

from math import comb

import ml_dtypes
import numpy as np

BATCH = 16384
IN_F = 512
OUT_F = 512
N_CORES = 8
BS = BATCH // N_CORES        # 2048 batch rows per core
BT = 512                     # moving-dim (batch) tile
NB = BS // BT                # 4 batch tiles per core
NFB = IN_F // 128            # 4 feature blocks
NQ = 5                       # basis groups per feature: x, x2, x3, R6, R7
KT = NFB * NQ                # 20 contraction k-tiles of 128
NO = OUT_F // 128            # 4 output blocks

_CACHE = {}


def _col_coeffs():
    # Coefficients of spline columns j=0..7 over {1, d, d2, d3, R6, R7},
    # d = s - 6.75, s = 2.5x + 5.5.
    a = [1.0, -4.0, 6.0, -4.0, 1.0]
    C = np.zeros((8, 6))
    for j in range(8):
        m = np.zeros(4)
        for k in range(5):
            p = j + k
            if p <= 5:
                e = 6.75 - p
                m += (a[k] / 6.0) * np.array([e**3, 3 * e**2, 3 * e, 1.0])
        C[j, :4] = m
        if 0 <= 6 - j <= 4:
            C[j, 4] = a[6 - j] / 6.0
        if 0 <= 7 - j <= 4:
            C[j, 5] = a[7 - j] / 6.0
    return C


def _prep_weights(base_weight, spline_weight, spline_scaler):
    C = _col_coeffs()
    # change of basis {1, d, d2, d3} -> x-monomials {1, x, x2, x3}:
    # d^m = sum_j binom(m,j) (2.5x)^j (-1.25)^(m-j)
    T = np.zeros((4, 4))
    for m in range(4):
        for j in range(m + 1):
            T[m, j] = comb(m, j) * (2.5**j) * ((-1.25) ** (m - j))
    Cx = np.zeros((8, 6))
    Cx[:, :4] = C[:, :4] @ T
    Cx[:, 4:] = C[:, 4:]
    W = spline_weight.astype(np.float64) * spline_scaler.astype(np.float64)[:, :, None]
    Wt = np.einsum("ofj,jq->ofq", W, Cx)         # (out, in, 6) over {1,x,x2,x3,R6,R7}
    # Fold the base branch in as well: silu on [0,1) fitted (max err 1.7e-5)
    # in the same 6-function span.
    xs = np.linspace(0, 1, 8193)[:-1]
    V = np.stack([np.ones_like(xs), xs, xs**2, xs**3,
                  np.maximum(2.5 * xs - 0.5, 0) ** 3,
                  np.maximum(2.5 * xs - 1.5, 0) ** 3], -1)
    coef = np.linalg.lstsq(V, xs / (1 + np.exp(-xs)), rcond=None)[0]
    Wt = Wt + base_weight.astype(np.float64)[:, :, None] * coef[None, None, :]
    bias = Wt[:, :, 0].sum(axis=1)               # (out,)
    # per-fb weight block: [128 in-features, NQ*OUT_F] with q-major columns,
    # one contiguous 640 KiB DMA per fb. Group order: x, x2, x3, R6, R7.
    wT = np.empty((NFB, 128, NQ * OUT_F), dtype=ml_dtypes.bfloat16)
    for fb in range(NFB):
        fs = slice(fb * 128, (fb + 1) * 128)
        for q in range(NQ):
            wT[fb, :, q * OUT_F:(q + 1) * OUT_F] = \
                Wt[:, fs, q + 1].T.astype(ml_dtypes.bfloat16)
    # (128, NO): column ob holds the biases for out-features ob*128..+128
    return wT, np.ascontiguousarray(bias.astype(np.float32).reshape(NO, 128).T)


def _build_program():
    if "nc" in _CACHE:
        return _CACHE["nc"]
    import concourse.bacc as bacc
    import concourse.mybir as mybir
    import concourse.tile as tile

    f32 = mybir.dt.float32
    bf16 = mybir.dt.bfloat16
    AF = mybir.ActivationFunctionType
    ALU = mybir.AluOpType

    nc = bacc.Bacc(None, target_bir_lowering=False, debug=False, num_devices=N_CORES)
    xT_d = nc.dram_tensor("xT", (NFB, 128, BS), bf16, kind="ExternalInput")
    wT_d = nc.dram_tensor("wT", (NFB, 128, NQ * OUT_F), bf16, kind="ExternalInput")
    bias_d = nc.dram_tensor("bias", (128, NO), f32, kind="ExternalInput")
    outT_d = nc.dram_tensor("outT", (OUT_F, BS), f32, kind="ExternalOutput")

    with tile.TileContext(nc) as tc:
        with (
            tc.tile_pool(name="wpool", bufs=1) as wpool,
            tc.tile_pool(name="bpool", bufs=32) as bpool,
            tc.tile_pool(name="spool", bufs=8) as spool,
            tc.tile_pool(name="opool", bufs=8) as opool,
            tc.tile_pool(name="psum", bufs=2, space="PSUM") as ppool,
        ):
            # Dummy warm-up matmuls bridge the gap between the PE's preamble
            # (~6.5us, all-engine barriers) and the first weights landing
            # (~9.7us): they keep the HAM activity window busy so the real
            # stream runs at 2.4 GHz from its first instruction.
            # The memset rides gpsimd (free until its DMA triggers), so the
            # dummies dispatch moments after the PE finishes its preamble.
            dummy_sb = wpool.tile([128, BT], bf16, tag="dummy")
            nc.gpsimd.memset(dummy_sb[:], 0.0)
            dummy_ps = ppool.tile([128, BT], f32, tag="acc0", name="dummy_ps")
            for _ in range(8):
                nc.tensor.matmul(dummy_ps[:], dummy_sb[:, 0:128], dummy_sb[:],
                                 start=True, stop=True)

            # Input DMAs spread across all three provisioned queues (sync
            # HWDGE, scalar HWDGE, gpsimd SWDGE; ~165 GB/s each, HBM-capped
            # in aggregate), ordered so each block lands just before the
            # PE/DVE needs it. fb-major k-order means the critical prefix is
            # x_fb0(bt0 slice) + w_fb0 cols 0:1536.
            x_sb = [None] * NFB
            w_sb = [None] * NFB
            for fb in range(NFB):
                x_sb[fb] = wpool.tile([128, BS], bf16, tag=f"x{fb}", name=f"x{fb}")
                w_sb[fb] = wpool.tile([128, NQ * OUT_F], bf16, tag=f"w{fb}",
                                      name=f"w{fb}")
            # head uses only sync + gpsimd (a third concurrent queue measured
            # SLOWER in aggregate); scalar HWDGE only carries tail outputs.
            nc.sync.dma_start(w_sb[0][:], wT_d[0])
            nc.sync.dma_start(w_sb[1][:], wT_d[1])
            nc.sync.dma_start(x_sb[0][:, BT:], xT_d[0][:, BT:])
            nc.sync.dma_start(x_sb[3][:], xT_d[3])
            nc.gpsimd.dma_start(x_sb[0][:, 0:BT], xT_d[0][:, 0:BT])
            nc.gpsimd.dma_start(x_sb[1][:], xT_d[1])
            nc.gpsimd.dma_start(x_sb[2][:], xT_d[2])
            nc.gpsimd.dma_start(w_sb[2][:], wT_d[2])
            nc.gpsimd.dma_start(w_sb[3][:], wT_d[3])
            bias_sb = wpool.tile([128, NO], f32, tag="bias")
            nc.gpsimd.dma_start(bias_sb[:], bias_d[:])
            cbias = {}
            for v in (-0.5, -1.5):
                ct = wpool.tile([128, 1], f32, tag=f"c{v}")
                nc.vector.memset(ct[:], v)
                cbias[v] = ct

            for bt in range(NB):
                bsl = slice(bt * BT, (bt + 1) * BT)
                basis = [None] * KT          # k = fb*NQ + q
                for fb in range(NFB):
                    xt = x_sb[fb][:, bsl]
                    x2 = bpool.tile([128, BT], bf16, tag="basis")
                    x3 = bpool.tile([128, BT], bf16, tag="basis")
                    r6 = bpool.tile([128, BT], bf16, tag="basis")
                    r7 = bpool.tile([128, BT], bf16, tag="basis")
                    u6 = spool.tile([128, BT], bf16, tag="tmp")
                    u7 = spool.tile([128, BT], bf16, tag="tmp")
                    q6 = spool.tile([128, BT], bf16, tag="tmp")
                    q7 = spool.tile([128, BT], bf16, tag="tmp")
                    # ACT: the two relu shoulders u = relu(2.5x + b)
                    nc.scalar.activation(u6[:], xt, AF.Relu, scale=2.5,
                                         bias=cbias[-0.5][:])
                    nc.scalar.activation(u7[:], xt, AF.Relu, scale=2.5,
                                         bias=cbias[-1.5][:])
                    # DVE: pure bf16 mul chains; relu(u)^3 = relu(u)^2*relu(u)
                    nc.vector.tensor_mul(x2[:], xt, xt)
                    nc.vector.tensor_mul(x3[:], x2[:], xt)
                    nc.vector.tensor_mul(q6[:], u6[:], u6[:])
                    nc.vector.tensor_mul(r6[:], q6[:], u6[:])
                    nc.vector.tensor_mul(q7[:], u7[:], u7[:])
                    nc.vector.tensor_mul(r7[:], q7[:], u7[:])
                    grp = [xt, x2[:], x3[:], r6[:], r7[:]]
                    for q in range(NQ):
                        basis[fb * NQ + q] = grp[q]
                def mm(k, ob, acc, csl=slice(0, BT)):
                    fb, q = divmod(k, NQ)
                    nc.tensor.matmul(
                        acc[:],
                        w_sb[fb][:, q * OUT_F + ob * 128:q * OUT_F + ob * 128 + 128],
                        basis[k][:, csl],
                        start=(k == 0), stop=(k == KT - 1),
                    )

                if bt < NB - 1:
                    # k-major: k=0 needs only the x_fb0 + w_fb0 DMAs
                    accs = []
                    for ob in range(NO):
                        acc = ppool.tile([128, BT], f32, tag=f"acc{ob}",
                                         name=f"acc{ob}")
                        accs.append(acc)
                    for k in range(KT):
                        for ob in range(NO):
                            mm(k, ob, accs[ob])
                    for ob in range(NO):
                        osl = slice(ob * 128, (ob + 1) * 128)
                        ot = opool.tile([128, BT], f32, tag="o")
                        nc.vector.tensor_scalar(ot[:], accs[ob][:],
                                                bias_sb[:, ob:ob + 1], None, ALU.add)
                        nc.sync.dma_start(outT_d[osl, bsl], ot[:])
                else:
                    # ob-major on the last tile: acc[ob] stops 20 MMs before
                    # acc[ob+1], so evictions overlap the remaining matmuls;
                    # the final ob runs in two batch halves to shorten the
                    # last evict+writeback chain, split across both HWDGE
                    # queues.
                    for ob in range(NO - 1):
                        acc = ppool.tile([128, BT], f32, tag=f"acc{ob}",
                                         name=f"acc{ob}")
                        for k in range(KT):
                            mm(k, ob, acc)
                        osl = slice(ob * 128, (ob + 1) * 128)
                        ot = opool.tile([128, BT], f32, tag="o")
                        nc.vector.tensor_scalar(ot[:], acc[:],
                                                bias_sb[:, ob:ob + 1], None, ALU.add)
                        eng = nc.sync if ob % 2 == 0 else nc.scalar
                        eng.dma_start(outT_d[osl, bsl], ot[:])
                    ob = NO - 1
                    for h in range(2):
                        acch = ppool.tile([128, BT // 2], f32, tag=f"acc{ob}",
                                          name=f"acc{ob}h{h}")
                        csl = slice(h * (BT // 2), (h + 1) * (BT // 2))
                        for k in range(KT):
                            mm(k, ob, acch, csl)
                        oth = opool.tile([128, BT // 2], f32, tag="o",
                                         name=f"ot{ob}h{h}")
                        nc.vector.tensor_scalar(oth[:], acch[:],
                                                bias_sb[:, ob:ob + 1], None, ALU.add)
                        eng = nc.sync if h == 0 else nc.scalar
                        eng.dma_start(
                            outT_d[ob * 128:(ob + 1) * 128,
                                   bt * BT + h * (BT // 2):
                                   bt * BT + (h + 1) * (BT // 2)],
                            oth[:])

    nc.compile()
    _CACHE["nc"] = nc
    return nc


def _make_in_maps(x, base_weight, spline_weight, spline_scaler):
    wT, bias = _prep_weights(base_weight, spline_weight, spline_scaler)
    in_maps = []
    for c in range(N_CORES):
        xs = np.ascontiguousarray(
            x[c * BS:(c + 1) * BS, :].T
        ).reshape(NFB, 128, BS).astype(ml_dtypes.bfloat16)
        in_maps.append({"xT": xs, "wT": wT, "bias": bias})
    return in_maps


def kernel(x, base_weight, spline_weight, spline_scaler):
    from concourse.bass_utils import run_bass_kernel_spmd

    nc = _build_program()
    in_maps = _make_in_maps(x, base_weight, spline_weight, spline_scaler)
    res = run_bass_kernel_spmd(nc, in_maps, list(range(N_CORES)))
    out = np.empty((BATCH, OUT_F), dtype=np.float32)
    for c in range(N_CORES):
        out[c * BS:(c + 1) * BS, :] = res.results[c]["outT"].T
    return out
